# revision 2
# baseline (speedup 1.0000x reference)
"""Expert-parallel MoE (top-2 routing) for 8 Trainium2 NeuronCores.

Strategy (hardcoded for the nn_MoE_28097676051036 problem shapes):
  - Host (numpy, float64): gating softmax, top-2 selection, gate
    normalization, aux loss, and token->expert dispatch (gather).
  - Device (one expert per core, SPMD over 8 cores): the two FFN matmuls
    h = relu(x @ w1 + b1), y_e = gate * (h @ w2), computed in float32r
    (full-rate 4-byte matmul mode) with fp32 PSUM accumulation.
  - Host: scatter-add per-expert outputs back into the token axis and add
    the (gate-weighted) b2 contribution.

Problem shapes: x[4,2048,512] f32, w_gate[8,512], w1[8,512,1024],
b1[8,1024], w2[8,1024,512], b2[8,512]; N=8192 tokens, top-2 of 8 experts.
"""

import numpy as np

K = 2
EPS = 1e-6
CVLOSS = 0.0
SWITCHLOSS = 0.01
ZLOSS = 0.001

B, L, D, H, E = 4, 2048, 512, 1024, 8
N = B * L
NCORES = 8
P = 128

# relu placement: "act" = ScalarE activation, "dve" = VectorE tensor_scalar
RELU_ENGINE = "act"

_RUNNER_CACHE: dict = {}


def _build_bass(C: int):
    """Build the per-core Bass module: FFN for one expert over C (padded)
    dispatched tokens. Inputs are laid out for the PE array:
      xt [D, C]  : gathered tokens, transposed (f32 bits, used as f32r)
      w1 [D, H], w2 [H, D], b1 [H, 1], g [C, 1]
    Output y [C, D] = g * relu(xt.T @ w1 + b1) @ w2   (b2 added on host).
    """
    import concourse.mybir as mybir
    import concourse.tile as tile
    from concourse import bacc

    dtr = mybir.dt.float32r
    dtf = mybir.dt.float32

    nc = bacc.Bacc()
    xt = nc.dram_tensor("xt", [D, C], dtr, kind="ExternalInput")
    w1 = nc.dram_tensor("w1", [D, H], dtr, kind="ExternalInput")
    b1 = nc.dram_tensor("b1", [H, 1], dtf, kind="ExternalInput")
    w2 = nc.dram_tensor("w2", [H, D], dtr, kind="ExternalInput")
    g = nc.dram_tensor("g", [C, 1], dtf, kind="ExternalInput")
    y = nc.dram_tensor("y", [C, D], dtf, kind="ExternalOutput")

    xt_t = xt.rearrange("(t p) c -> t p c", p=P)  # [KD, 128, C]
    w1_t = w1.rearrange("(t p) h -> t p h", p=P)  # [KD, 128, H]
    w2_t = w2.rearrange("(t p) d -> t p d", p=P)  # [KH, 128, D]
    b1_t = b1.rearrange("(t p) o -> t p o", p=P)  # [KH, 128, 1]
    g_t = g.rearrange("(t p) o -> t p o", p=P)  # [CT, 128, 1]
    y_t = y.rearrange("(t p) d -> t p d", p=P)  # [CT, 128, D]

    KD = D // P  # 4 k-tiles for the first matmul
    KH = H // P  # 8 k-tiles for the second matmul
    CT = C // P  # token tiles

    # free-dim chunks for stage A (moving operand max 512 for 4-byte dtypes;
    # f32r needs >=256 free dim for full rate)
    chunks = []
    c0 = 0
    while c0 < C:
        w = min(512, C - c0)
        chunks.append((c0, w))
        c0 += w

    with tile.TileContext(nc) as tc:
        with (
            tc.tile_pool(name="weights", bufs=1) as wpool,
            tc.tile_pool(name="data", bufs=1) as dpool,
            tc.tile_pool(name="out", bufs=4) as opool,
            tc.tile_pool(name="psA", bufs=4, space="PSUM") as psA,
            tc.tile_pool(name="psB", bufs=4, space="PSUM") as psB,
        ):
            w1_sb = [wpool.tile([P, H], dtr, name=f"w1_{i}") for i in range(KD)]
            w2_sb = [wpool.tile([P, D], dtr, name=f"w2_{i}") for i in range(KH)]
            b1_sb = [wpool.tile([P, 1], dtf, name=f"b1_{i}") for i in range(KH)]
            xt_sb = [dpool.tile([P, C], dtr, name=f"xt_{i}") for i in range(KD)]
            g_sb = [dpool.tile([P, 1], dtf, name=f"g_{i}") for i in range(CT)]
            hT_sb = [dpool.tile([P, C], dtr, name=f"hT_{i}") for i in range(KH)]

            for i in range(KD):
                nc.sync.dma_start(out=w1_sb[i][:], in_=w1_t[i])
                nc.sync.dma_start(out=xt_sb[i][:], in_=xt_t[i])
            for i in range(KH):
                nc.sync.dma_start(out=w2_sb[i][:], in_=w2_t[i])
                nc.sync.dma_start(out=b1_sb[i][:], in_=b1_t[i])
            for i in range(CT):
                nc.sync.dma_start(out=g_sb[i][:], in_=g_t[i])

            # Stage A: hT[h-tile] = relu(w1.T-slice @ xg + b1), laid out
            # [h partitions, token free dim]. Chunk-outer so stage B can
            # start on early token tiles while later chunks still compute.
            for c0, cw in chunks:
                for h in range(KH):
                    ps = psA.tile([P, 512], mybir.dt.float32, name="psa")
                    for kd in range(KD):
                        nc.tensor.matmul(
                            ps[:, :cw],
                            w1_sb[kd][:, h * P : (h + 1) * P],
                            xt_sb[kd][:, c0 : c0 + cw],
                            start=(kd == 0),
                            stop=(kd == KD - 1),
                        )
                    if RELU_ENGINE == "act":
                        nc.scalar.activation(
                            hT_sb[h][:, c0 : c0 + cw],
                            ps[:, :cw],
                            mybir.ActivationFunctionType.Relu,
                            bias=b1_sb[h][:, 0:1],
                        )
                    else:
                        nc.vector.tensor_scalar(
                            hT_sb[h][:, c0 : c0 + cw],
                            ps[:, :cw],
                            b1_sb[h][:, 0:1],
                            0.0,
                            mybir.AluOpType.add,
                            mybir.AluOpType.max,
                        )

            # Stage B: y[c-tile] = g * (hT-slice.T @ w2)
            for ct in range(CT):
                ps2 = psB.tile([P, D], mybir.dt.float32, name="psb")
                for h in range(KH):
                    nc.tensor.matmul(
                        ps2[:],
                        hT_sb[h][:, ct * P : (ct + 1) * P],
                        w2_sb[h][:],
                        start=(h == 0),
                        stop=(h == KH - 1),
                    )
                yt = opool.tile([P, D], dtf, name="yt")
                nc.vector.tensor_scalar_mul(yt[:], ps2[:], g_sb[ct][:, 0:1])
                nc.sync.dma_start(out=y_t[ct], in_=yt[:])

    nc.finalize()
    return nc


def _make_runner(C: int):
    """Compile the Bass module once and return a callable
    run(per_core_in_maps) -> list of per-core output dicts.
    Mirrors concourse.bass2jax.run_bass_via_pjrt but caches the jitted
    executable across invocations."""
    import jax
    import concourse.mybir as mybir
    from concourse import bass2jax
    from jax.experimental.shard_map import shard_map
    from jax.sharding import Mesh, PartitionSpec

    nc = _build_bass(C)
    bass2jax.install_neuronx_cc_hook()

    partition_name = nc.partition_id_tensor.name if nc.partition_id_tensor else None

    in_names = []
    out_names = []
    out_avals = []
    out_shapes = []
    for alloc in nc.m.functions[0].allocations:
        if not isinstance(alloc, mybir.MemoryLocationSet):
            continue
        name = alloc.memorylocations[0].name
        if alloc.kind == "ExternalInput":
            if name != partition_name:
                in_names.append(name)
        elif alloc.kind == "ExternalOutput":
            shape = tuple(alloc.tensor_shape)
            dtype = mybir.dt.np(alloc.dtype)
            out_names.append(name)
            out_avals.append(jax.core.ShapedArray(shape, dtype))
            out_shapes.append((shape, dtype))
    n_params = len(in_names)
    n_outs = len(out_names)
    all_in_names = in_names + out_names
    if partition_name is not None:
        all_in_names = all_in_names + [partition_name]

    donate = tuple(range(n_params, n_params + n_outs))

    def _body(*args):
        operands = list(args)
        if partition_name is not None:
            operands.append(bass2jax.partition_id_tensor())
        outs = bass2jax._bass_exec_p.bind(
            *operands,
            out_avals=tuple(out_avals),
            in_names=tuple(all_in_names),
            out_names=tuple(out_names),
            lowering_input_output_aliases=(),
            sim_require_finite=True,
            sim_require_nnan=True,
            nc=nc,
        )
        return tuple(outs)

    devices = jax.devices()[:NCORES]
    mesh = Mesh(np.asarray(devices), ("core",))
    in_specs = (PartitionSpec("core"),) * (n_params + n_outs)
    out_specs = (PartitionSpec("core"),) * n_outs
    sharded = jax.jit(
        shard_map(
            _body, mesh=mesh, in_specs=in_specs, out_specs=out_specs, check_rep=False
        ),
        donate_argnums=donate,
        keep_unused=True,
    )

    def run(per_core_in_maps):
        concat_in = [
            np.concatenate([m[name] for m in per_core_in_maps], axis=0)
            for name in in_names
        ]
        concat_zeros = [
            np.zeros((NCORES * s[0], *s[1:]), dt) for (s, dt) in out_shapes
        ]
        out_arrs = sharded(*concat_in, *concat_zeros)
        return [
            {
                name: np.asarray(out_arrs[i]).reshape(
                    NCORES, *out_shapes[i][0]
                )[c]
                for i, name in enumerate(out_names)
            }
            for c in range(NCORES)
        ]

    return run


def _get_runner(C: int):
    if C not in _RUNNER_CACHE:
        _RUNNER_CACHE[C] = _make_runner(C)
    return _RUNNER_CACHE[C]


def _routing(xf, w_gate):
    """Host-side gating in float64: returns top-2 indices [N,2], normalized
    gates [N,2] (f32), and the aux loss (f64 scalar)."""
    logits = xf.astype(np.float64) @ w_gate.astype(np.float64).T  # [N, E]
    m = logits.max(axis=1, keepdims=True)
    ex = np.exp(logits - m)
    sex = ex.sum(axis=1, keepdims=True)
    probs = ex / sex  # [N, E]

    idx = np.argsort(-probs, axis=1, kind="stable")[:, :K]  # [N, 2]
    rows = np.arange(N)[:, None]
    topg = probs[rows, idx]  # [N, 2]
    denom = topg.sum(axis=1, keepdims=True) + EPS
    gates = topg / denom  # [N, 2]

    # aux loss (CVLOSS term is multiplied by 0.0 in the reference)
    psum = probs.sum(axis=0)
    psum_n = psum / psum.sum()
    freqs = np.bincount(idx.ravel(), minlength=E).astype(np.float64)
    freqs_n = freqs / freqs.sum()
    switch = (psum_n * freqs_n).sum() * E
    lse = np.log(sex[:, 0]) + m[:, 0]
    zl = np.mean(lse**2)
    loss = SWITCHLOSS * switch + ZLOSS * zl
    if CVLOSS != 0.0:
        cv = np.var(psum_n, ddof=1) / (np.mean(psum_n) ** 2 + 1e-10)
        loss += CVLOSS * cv
    return idx, gates.astype(np.float32), loss


def kernel(x, w_gate, w1, b1, w2, b2):
    x = np.asarray(x, dtype=np.float32)
    w_gate = np.asarray(w_gate, dtype=np.float32)
    w1 = np.asarray(w1, dtype=np.float32)
    b1 = np.asarray(b1, dtype=np.float32)
    w2 = np.asarray(w2, dtype=np.float32)
    b2 = np.asarray(b2, dtype=np.float32)

    xf = x.reshape(N, D)
    idx, gates, loss = _routing(xf, w_gate)

    # dispatch: group (token, gate) pairs by expert
    flat_e = idx.ravel()
    flat_tok = np.repeat(np.arange(N), K)
    flat_g = gates.ravel()
    order = np.argsort(flat_e, kind="stable")
    counts = np.bincount(flat_e, minlength=E)
    offs = np.concatenate([[0], np.cumsum(counts)])

    C = int(np.ceil(counts.max() / P) * P)
    if C % 512 == P:  # avoid a 128-wide tail chunk (f32r slow below 256)
        C += P

    toks = []
    in_maps = []
    for e in range(E):
        sel = order[offs[e] : offs[e + 1]]
        te = flat_tok[sel]
        ge = flat_g[sel]
        toks.append(te)
        cnt = len(te)
        xg_T = np.zeros((D, C), dtype=np.float32)
        xg_T[:, :cnt] = xf[te].T
        g_arr = np.zeros((C, 1), dtype=np.float32)
        g_arr[:cnt, 0] = ge
        in_maps.append(
            {
                "xt": xg_T,
                "w1": np.ascontiguousarray(w1[e]),
                "b1": np.ascontiguousarray(b1[e].reshape(H, 1)),
                "w2": np.ascontiguousarray(w2[e]),
                "g": g_arr,
            }
        )

    run = _get_runner(C)
    outs = run(in_maps)

    y = np.zeros((N, D), dtype=np.float32)
    for e in range(E):
        cnt = len(toks[e])
        y[toks[e]] += outs[e]["y"][:cnt]

    # b2 contribution: y += sum_e gates_dense[:, e] * b2[e]
    gates_dense = np.zeros((N, E), dtype=np.float32)
    gates_dense[np.arange(N)[:, None], idx] = gates
    y += gates_dense @ b2

    return y.reshape(B, L, D), np.float32(loss)
